# revision 39
# baseline (speedup 1.0000x reference)
"""CG coupler (segment_reduce) Trainium2 kernel.

out[b, ro[t]] += x1[b, r1[t]] * x2[b, r2[t]] * cg[t]   for t in range(T)

The CG index tables produced by the coupler have a rigid structure: T splits
into runs of exactly 128 consecutive indices (the channel dimension) that are
128-aligned in all three tensors, with a constant coefficient per run.  Each
run is therefore one dense slot-level FMA:

    out[:, so*128:(so+1)*128] += c * x1[:, s1*128:...] * x2[:, s2*128:...]

We detect that structure from the runtime index arrays on the host and bake it
into the Bass program.  Per core (batch is data-parallel across 8 cores, no
collectives):

  - x1/x2 are converted to bf16 on the HOST, halving load DMA (the output
    stays fp32); chunks stream straight into the big bf16 tiles with
    fine-grained leading chunks (2-slot minimum: bf16 single-slot rows are
    256 B, below the 512 B descriptor latency cliff)
  - the distinct (s1,s2) slot products run as bf16 tensor_tensor ops split
    between DVE (2x_1p) and Pool (no DVE perf modes) by a host-side list
    scheduler
  - a few mirrored pairs are folded: c*pr_ab + (+-c)*pr_ba = c*(pr_ab +-
    pr_ba), trading one producer op for one fewer PE matmul
  - per-term scaled-identity bf16 matmuls (1 PE cycle/row) accumulate into
    PSUM; each output slot's matmuls form one contiguous start/stop group,
    so no PSUM-zeroing matmuls are needed
  - Act evacuates each PSUM bank to SBUF as soon as its two slots finish;
    the bank's output columns are then DMA'd to DRAM immediately
  - every instruction is emitted into one globally ordered stream
    (estimated-start-time heap gated by dependencies), because the Tile
    framework derives dependencies from program order

Cost-model notes that shaped this (from bass_rust instruction_cost_v2):
fp32 matmul = 4 PE cycles/row, float32r >= 256 rows = 1, bf16 = 1;
DVE 2x/4x perf modes apply only to the DVE engine, never Pool; LdWeights
is free; DMA is one exclusive device at ~360 GB/s aggregate, so per-core
HBM traffic (8.4 MB with bf16 inputs) floors DMA busy at ~23 us.
"""
import sys

for _p in ("/opt/trn_rl_repo",):
    if _p not in sys.path:
        sys.path.insert(0, _p)

from contextlib import ExitStack

import numpy as np

import concourse.bass as bass
import concourse.mybir as mybir
import concourse.tile as tile
from concourse import bacc
from concourse.bass_utils import run_bass_kernel_spmd

N_CORES = 8
P = 128
F32 = mybir.dt.float32
F32R = mybir.dt.float32r
BF16 = mybir.dt.bfloat16

_CACHE: dict = {}


def _detect_plan(r1, r2, ro, cg, in_dim, out_dim):
    """Return list of (s1, s2, so, c) slot terms, or None if the index tables
    don't have the aligned 128-run structure."""
    T = len(cg)
    if T % P != 0 or len(r1) != T or len(r2) != T or len(ro) != T:
        return None
    d1 = np.diff(r1)
    d2 = np.diff(r2)
    do = np.diff(ro)
    brk = np.where(~((d1 == 1) & (d2 == 1) & (do == 1)))[0] + 1
    starts = np.concatenate([[0], brk])
    ends = np.concatenate([brk, [T]])
    if not np.all(ends - starts == P):
        return None
    a0, b0, o0 = r1[starts], r2[starts], ro[starts]
    if (a0 % P).any() or (b0 % P).any() or (o0 % P).any():
        return None
    if a0.max() + P > in_dim or b0.max() + P > in_dim or o0.max() + P > out_dim:
        return None
    cg2 = np.asarray(cg).reshape(-1, P)
    if not np.all(cg2 == cg2[:, :1]):
        return None
    return list(
        zip(
            (a0 // P).tolist(),
            (b0 // P).tolist(),
            (o0 // P).tolist(),
            cg2[:, 0].astype(np.float64).tolist(),
        )
    )


def _numpy_fallback(x1, x2, cg, r1, r2, ro, out_dim):
    out = np.zeros((x1.shape[0], out_dim), dtype=x1.dtype)
    prod = x1[:, r1] * x2[:, r2] * cg[None, :].astype(x1.dtype)
    np.add.at(out, (slice(None), ro), prod)
    return out


# cost-model engine-busy estimates (ns) for [128, N]-free elementwise ops
def _dve_tt(free):  # bf16 tensor_tensor, 2x_1p (+ measured per-op overhead)
    return free * 1.0417 * 0.5 + 80.0


def _pool_tt(free):  # tensor_tensor; Pool gets no DVE 2x modes, 0.42 sw eff
    return free * 0.8333 / 0.42 + 30.0


def _dve_conv(free):  # fp32->bf16 tensor_copy, 2x_2p
    return free * 1.0417 * 0.5 + 60.0


def _act_conv(free):  # fp32->bf16 activation copy
    return free * 0.8333 + 185.0


def _pool_conv(free):  # fp32->bf16 copy on gpsimd (0.6 default sw efficiency)
    return free * 0.8333 / 0.6 + 30.0


_PLAN_CFG = {
    "act_vt0": 2200.0,
    "dve_conv_shadow": 1.0,
    "pool_conv": True,
    "work_conserve": False,  # producers: prefer the idle engine
    "act_conv_ps1": False,  # force pass>=1 conversions onto Act
    "chunks0": [2, 2, 2, 2, 4, 4],  # load chunk sizes (slots)
    "chunks1": [4, 4, 2, 2, 2, 2],  # later passes
    "act_conv_ps0_from": 3,  # pass-0 chunks >= this index convert on Act
    "n_combine": 10,  # mirrored pairs folded into S+- combines
    "act_conv_ps1_from": 99,  # pass>=1 chunks >= this index convert on Act
    "comb_offset": 2,  # skip the first N mirrored pairs when choosing combines
    "sid_spread": 40.0,  # est spacing between scaled-identity builds on Act
    "pool_evacs": 0,  # how many of the latest-finishing bank evacs go to Pool
    "n_combine_late": 0,  # also fold the latest-arriving mirrored pairs
}


_ACT_SID_NS = 292.0
_ACT_EVAC_NS = 612.0
_MM_NS = 107.0  # bf16 matmul, 256 moving rows

SLOTS_PER_GROUP = 4  # column-group granularity for input DMA (512 cols)


def _mirror_plan(pairs):
    """Split terms into direct terms and mirror-combined terms.

    Returns (direct, combined, combines) where
      direct:   list of (pair, so, c)             -> rhs = product(pair)
      combined: list of (upair, sign, so, c)      -> rhs = S_sign(upair)
      combines: list of (upair, sign)             -> S_sign = pr_ab + sign*pr_ba
    """
    direct, combined, combines = [], [], set()
    done = set()
    for (a, b), tl in pairs.items():
        if (a, b) in done:
            continue
        if a == b or (b, a) not in pairs:
            done.add((a, b))
            for so, c in tl:
                direct.append(((a, b), so, c))
            continue
        d1 = dict(tl)
        d2 = dict(pairs[(b, a)])
        done.add((a, b))
        done.add((b, a))
        if set(d1) != set(d2):
            for so, c in d1.items():
                direct.append(((a, b), so, c))
            for so, c in d2.items():
                direct.append(((b, a), so, c))
            continue
        ok = all(abs(abs(d1[so]) - abs(d2[so])) <= 1e-5 * abs(d1[so]) for so in d1)
        if not ok:
            for so, c in d1.items():
                direct.append(((a, b), so, c))
            for so, c in d2.items():
                direct.append(((b, a), so, c))
            continue
        up = (a, b) if a < b else (b, a)
        da, db = (d1, d2) if a < b else (d2, d1)
        for so in da:
            sign = 1 if da[so] * db[so] > 0 else -1
            combined.append((up, sign, so, da[so]))
            combines.add((up, sign))
    return direct, combined, sorted(combines)


def _build_program(terms, b_shard, in_dim, out_dim):
    """Build the per-core Bass program. Every core runs the same program on
    its own batch shard (data-parallel, no collectives).

    Slot-pass structure: all 4 batch row-blocks are in flight at once, so a
    PSUM bank holds exactly one output slot (512-wide moving dim) and each
    pair product is computed ONCE ([128, 512] bf16 tensor_tensor). The 16
    output slots run as two wavefronts of 8 banks; the second wave only
    waits for first-wave evacuations, not for new products.
    """
    nblk = b_shard // P
    assert nblk % 4 == 0 and nblk == 4, "slot-pass layout assumes 4 row-blocks"
    n_so = out_dim // P
    n_s_in = in_dim // P

    def chunks():
        sizes = _PLAN_CFG["chunks0"]
        out, s = [], 0
        for sz in sizes:
            if s >= n_s_in:
                break
            e = min(s + sz, n_s_in)
            out.append(list(range(s, e)))
            s = e
        while s < n_s_in:
            e = min(s + SLOTS_PER_GROUP, n_s_in)
            out.append(list(range(s, e)))
            s = e
        return out

    chunk_list = chunks()
    chunk_idx = {}
    for ci, chunk in enumerate(chunk_list):
        for s in chunk:
            chunk_idx[s] = ci

    # serial-DMA load completion estimates (bf16, 2x latency below 512 B)
    load_done = {}
    t = 1400.0
    for ci, chunk in enumerate(chunk_list):
        elem = len(chunk) * P * 2
        dur = nblk * P * elem / 0.36 * (2.0 if elem < 512 else 1.0)
        t += dur
        load_done[(0, ci)] = t
        t += dur
        load_done[(1, ci)] = t

    pairs: dict = {}
    for s1, s2, so, c in terms:
        pairs.setdefault((s1, s2), []).append((so, c))

    def grp_w(p):
        return max(p[0], p[1])

    direct_all, combined_all, _ = _mirror_plan(pairs)
    n_comb = _PLAN_CFG["n_combine"]
    upairs = sorted({up for up, sign, so, c in combined_all},
                    key=lambda up: max(grp_w(up), grp_w((up[1], up[0]))))
    off = _PLAN_CFG["comb_offset"]
    chosen = set(upairs[off : off + n_comb])
    direct = list(direct_all)
    combined, combines = [], set()
    for up, sign, so, c in combined_all:
        if up in chosen:
            combined.append((up, sign, so, c))
            combines.add((up, sign))
        else:
            direct.append((up, so, c))
            direct.append(((up[1], up[0]), so, c if sign > 0 else -c))
    combines = sorted(combines)

    # list-schedule products and combines on DVE/Pool (512-wide ops)
    import heapq as _hq

    FREE = nblk * P  # 512
    vt = {"dve": 0.0, "pool": 300.0}
    done = {}
    assign = {}
    comb_of_prod = {}
    for up, sign in combines:
        for pp in (up, (up[1], up[0])):
            comb_of_prod.setdefault(("prod", pp), []).append(("comb", up, sign))
    comb_deps = {("comb", up, sign): 2 for up, sign in combines}
    comb_ready = {k: 0.0 for k in comb_deps}

    heap = []
    seq = 0
    for p in pairs:
        ready = max(load_done[(0, chunk_idx[p[0]])], load_done[(1, chunk_idx[p[1]])])
        seq += 1
        _hq.heappush(heap, (ready, seq, ("prod", p)))
    while heap:
        ready, _, key = _hq.heappop(heap)
        cand = [
            ("dve", max(ready, vt["dve"]) + _dve_tt(FREE), _dve_tt(FREE)),
            ("pool", max(ready, vt["pool"]) + _pool_tt(FREE), _pool_tt(FREE)),
        ]
        eng, fin, cost = min(cand, key=lambda c: c[1])
        fin = max(ready, vt[eng]) + cost
        vt[eng] = fin
        assign[key] = eng
        done[key] = fin
        if key[0] == "prod":
            for w in comb_of_prod.get(key, []):
                comb_ready[w] = max(comb_ready[w], fin)
                comb_deps[w] -= 1
                if comb_deps[w] == 0:
                    seq += 1
                    _hq.heappush(heap, (comb_ready[w], seq, w))

    # slot plan: one wavefront ordering by last-rhs completion; first 8 slots
    # get fresh banks, the rest alias the earliest-evacuating banks
    rhs_of = {}
    for p, so, c in direct:
        rhs_of.setdefault(so, []).append((("prod", p), c))
    for up, sign, so, c in combined:
        rhs_of.setdefault(so, []).append((("comb", up, sign), c))
    key_of = {so: max(done[rk] for rk, _ in tl) for so, tl in rhs_of.items()}
    slot_order = sorted(rhs_of, key=lambda so: (key_of[so], so))
    n_banks = 8
    bank_of = {}
    evac_est = {}
    cvals_first_use = {}
    pe_vt = 0.0
    _MMW = FREE * 0.4167  # bf16 matmul ns at 512 moving rows
    for i, so in enumerate(slot_order):
        tl = sorted(rhs_of[so], key=lambda rc: done[rc[0]])
        rhs_of[so] = tl
        for rk, c in tl:
            pe_vt = max(pe_vt, done[rk]) + _MMW
            cvals_first_use.setdefault(c, len(cvals_first_use))
        bank_of[so] = (
            i if i < n_banks else bank_of[slot_order[i - n_banks]]
        )
        # monotone in slot_order: PE retires groups in exactly this order,
        # so this emission order never head-of-line-blocks the Act queue
        evac_est[so] = max(pe_vt + 100.0,
                           (evac_est[slot_order[i - 1]] + 1.0) if i else 0.0)

    # --- emit -------------------------------------------------------------
    nc = bacc.Bacc("TRN2", target_bir_lowering=False, debug=False)
    x1d = nc.dram_tensor("x1", [b_shard, in_dim], BF16, kind="ExternalInput").ap()
    x2d = nc.dram_tensor("x2", [b_shard, in_dim], BF16, kind="ExternalInput").ap()
    outd = nc.dram_tensor("out", [b_shard, out_dim], F32, kind="ExternalOutput").ap()

    with tile.TileContext(nc) as tc, ExitStack() as ctx:
        const_p = ctx.enter_context(tc.tile_pool(name="const", bufs=1))
        big_p = ctx.enter_context(tc.tile_pool(name="big", bufs=1))
        prod_p = ctx.enter_context(tc.tile_pool(name="prod", bufs=1))
        psum_p = ctx.enter_context(tc.tile_pool(name="psum", bufs=8, space="PSUM"))

        ident = const_p.tile([P, P], F32, tag="ident")
        nc.gpsimd.memset(ident[:], 0.0)
        nc.gpsimd.affine_select(
            out=ident[:],
            in_=ident[:],
            compare_op=mybir.AluOpType.not_equal,
            fill=1.0,
            base=0,
            pattern=[[-1, P]],
            channel_multiplier=1,
        )

        X1B = big_p.tile([P, nblk * in_dim], BF16, tag="X1B")
        X2B = big_p.tile([P, nblk * in_dim], BF16, tag="X2B")
        OUT = big_p.tile([P, nblk * out_dim], F32, tag="OUT")
        XBr = [
            X1B[:].rearrange("p (blk f) -> p blk f", blk=nblk),
            X2B[:].rearrange("p (blk f) -> p blk f", blk=nblk),
        ]
        OUTr = OUT[:].rearrange("p (blk f) -> p blk f", blk=nblk)

        banks = []
        for k in range(2 * n_banks):
            bk = psum_p.tile([P, FREE], F32, tag="bank")
            banks.append(bk)
        bank_tile = {}
        fresh = 0
        for i, so in enumerate(slot_order):
            bank_tile[so] = banks[i]  # pool rotation: i>=8 aliases i-8

        sids = {}
        for c, i in sorted(cvals_first_use.items(), key=lambda kv: kv[1]):
            t_ = const_p.tile([P, P], BF16, tag=f"sid{i}")
            sids[c] = t_

        raw_events = []

        def add(eid, est, deps, emit):
            raw_events.append((eid, est, deps, emit))

        for c, i in sorted(cvals_first_use.items(), key=lambda kv: kv[1]):
            def em_sid(c=c):
                nc.scalar.activation(
                    out=sids[c][:],
                    in_=ident[:],
                    func=mybir.ActivationFunctionType.Copy,
                    scale=float(c),
                )
            add(("sid", c), 500.0 + _PLAN_CFG["sid_spread"] * i, [], em_sid)

        for ci, chunk in enumerate(chunk_list):
            cols = slice(chunk[0] * P, (chunk[-1] + 1) * P)
            for tn, xd in ((0, x1d), (1, x2d)):
                elem = len(chunk) * P * 2
                dur = nblk * P * elem / 0.36 * (2.0 if elem < 512 else 1.0)
                def em_load(tn=tn, cols=cols, xd=xd):
                    nc.sync.dma_start(
                        out=XBr[tn][:, :, cols],
                        in_=xd[:, cols].rearrange("(blk p) f -> p blk f", p=P),
                    )
                add(("load", ci, tn), load_done[(tn, ci)] - dur, [], em_load)

        tiles = {}
        for p in pairs:
            key = ("prod", p)
            deps = [("load", chunk_idx[p[0]], 0), ("load", chunk_idx[p[1]], 1)]
            eng_name = assign[key]
            def em_prod(p=p, eng_name=eng_name, key=key):
                pr = prod_p.tile([P, FREE], BF16, tag="prod", bufs=96)
                eng = nc.vector if eng_name == "dve" else nc.gpsimd
                eng.tensor_tensor(
                    out=pr[:].rearrange("p (b f) -> p b f", b=nblk),
                    in0=XBr[0][:, :, p[0] * P : (p[0] + 1) * P],
                    in1=XBr[1][:, :, p[1] * P : (p[1] + 1) * P],
                    op=mybir.AluOpType.mult,
                )
                tiles[key] = pr
            add(key, done[key] - _dve_tt(FREE), deps, em_prod)

        for up, sign in combines:
            key = ("comb", up, sign)
            eng_name = assign[key]
            def em_comb(up=up, sign=sign, eng_name=eng_name, key=key):
                pr = prod_p.tile([P, FREE], BF16, tag="prod", bufs=96)
                eng = nc.vector if eng_name == "dve" else nc.gpsimd
                eng.tensor_tensor(
                    out=pr[:].rearrange("p (b f) -> p b f", b=nblk),
                    in0=tiles[("prod", up)][:].rearrange("p (b f) -> p b f", b=nblk),
                    in1=tiles[("prod", (up[1], up[0]))][:].rearrange(
                        "p (b f) -> p b f", b=nblk
                    ),
                    op=mybir.AluOpType.add if sign > 0 else mybir.AluOpType.subtract,
                )
                tiles[key] = pr
            add(key, done[key] - _dve_tt(FREE),
                [("prod", up), ("prod", (up[1], up[0]))], em_comb)

        for i, so in enumerate(slot_order):
            tl = rhs_of[so]
            deps = [rk for rk, _ in tl] + [("sid", c) for _, c in tl]
            if i >= n_banks:
                deps.append(("evac", slot_order[i - n_banks]))
            deps = sorted(set(deps))
            def em_slot(so=so, tl=tl):
                for j, (rk, c) in enumerate(tl):
                    nc.tensor.matmul(
                        out=bank_tile[so][:],
                        lhsT=sids[c][:],
                        rhs=tiles[rk][:],
                        start=(j == 0),
                        stop=(j == len(tl) - 1),
                    )
            add(("slot", so), key_of[so], deps, em_slot)

            def em_evac(so=so):
                nc.scalar.copy(
                    out=OUTr[:, :, so * P : (so + 1) * P],
                    in_=bank_tile[so][:].rearrange("p (b f) -> p b f", b=nblk),
                )
            add(("evac", so), evac_est[so], [("slot", so)], em_evac)

            store_est = max(evac_est[so] + 650.0, max(load_done.values()) + 1.0)
            def em_store(so=so):
                nc.sync.dma_start(
                    out=outd[:, so * P : (so + 1) * P].rearrange(
                        "(blk p) f -> p blk f", p=P
                    ),
                    in_=OUTr[:, :, so * P : (so + 1) * P],
                )
            add(("store", so), store_est, [("evac", so)], em_store)

        # topological emission in estimated-start order
        import heapq
        events = {}
        dependents = {}
        for eid, est, deps, emit in raw_events:
            events[eid] = {"est": est, "deps": [], "emit": emit}
        for eid, est, deps, emit in raw_events:
            for d in deps:
                assert d in events, (eid, d)
                events[eid]["deps"].append(d)
                dependents.setdefault(d, []).append(eid)
        ndeps = {eid: len(ev["deps"]) for eid, ev in events.items()}
        heap2 = []
        ctr = 0
        for eid, ev in events.items():
            if ndeps[eid] == 0:
                heapq.heappush(heap2, (ev["est"], ctr, eid))
                ctr += 1
        emitted = 0
        while heap2:
            _, _, eid = heapq.heappop(heap2)
            events[eid]["emit"]()
            emitted += 1
            for dep in dependents.get(eid, []):
                ndeps[dep] -= 1
                if ndeps[dep] == 0:
                    heapq.heappush(heap2, (events[dep]["est"], ctr, dep))
                    ctr += 1
        assert emitted == len(events), (emitted, len(events))

    nc.finalize()  # run the bacc pass pipeline (wait splitting, regalloc, ...)
    return nc


def kernel(x1, x2, cg_tilde, repids_in1, repids_in2, repids_out, out_dim):
    x1 = np.ascontiguousarray(np.asarray(x1, dtype=np.float32))
    x2 = np.ascontiguousarray(np.asarray(x2, dtype=np.float32))
    cg = np.asarray(cg_tilde, dtype=np.float32)
    r1 = np.asarray(repids_in1).astype(np.int64)
    r2 = np.asarray(repids_in2).astype(np.int64)
    ro = np.asarray(repids_out).astype(np.int64)
    out_dim = int(np.asarray(out_dim))

    B, in_dim = x1.shape
    terms = None
    if (
        B % (N_CORES * 2 * P) == 0
        and in_dim % P == 0
        and out_dim % P == 0
        and x2.shape == x1.shape
    ):
        terms = _detect_plan(r1, r2, ro, cg, in_dim, out_dim)
    if terms is None:
        return _numpy_fallback(x1, x2, cg, r1, r2, ro, out_dim)

    b_shard = B // N_CORES
    key = (B, in_dim, out_dim, np.asarray(terms, dtype=np.float64).tobytes())
    nc = _CACHE.get(key)
    if nc is None:
        nc = _build_program(terms, b_shard, in_dim, out_dim)
        _CACHE[key] = nc

    import ml_dtypes

    x1b = x1.astype(ml_dtypes.bfloat16)
    x2b = x2.astype(ml_dtypes.bfloat16)
    in_maps = [
        {
            "x1": x1b[i * b_shard : (i + 1) * b_shard],
            "x2": x2b[i * b_shard : (i + 1) * b_shard],
        }
        for i in range(N_CORES)
    ]
    res = run_bass_kernel_spmd(nc, in_maps, core_ids=list(range(N_CORES)))
    return np.concatenate([res.results[i]["out"] for i in range(N_CORES)], axis=0)


# revision 40
# speedup vs baseline: 1.0021x; 1.0021x over previous
"""CG coupler (segment_reduce) Trainium2 kernel.

out[b, ro[t]] += x1[b, r1[t]] * x2[b, r2[t]] * cg[t]   for t in range(T)

The CG index tables produced by the coupler have a rigid structure: T splits
into runs of exactly 128 consecutive indices (the channel dimension) that are
128-aligned in all three tensors, with a constant coefficient per run.  Each
run is therefore one dense slot-level FMA:

    out[:, so*128:(so+1)*128] += c * x1[:, s1*128:...] * x2[:, s2*128:...]

We detect that structure from the runtime index arrays on the host and bake it
into the Bass program.  Per core (batch is data-parallel across 8 cores, no
collectives):

  - x1/x2 are converted to bf16 on the HOST, halving load DMA (the output
    stays fp32); chunks stream straight into the big bf16 tiles with
    fine-grained leading chunks (2-slot minimum: bf16 single-slot rows are
    256 B, below the 512 B descriptor latency cliff)
  - the distinct (s1,s2) slot products run as bf16 tensor_tensor ops split
    between DVE (2x_1p) and Pool (no DVE perf modes) by a host-side list
    scheduler
  - a few mirrored pairs are folded: c*pr_ab + (+-c)*pr_ba = c*(pr_ab +-
    pr_ba), trading one producer op for one fewer PE matmul
  - per-term scaled-identity bf16 matmuls (1 PE cycle/row) accumulate into
    PSUM; each output slot's matmuls form one contiguous start/stop group,
    so no PSUM-zeroing matmuls are needed
  - Act evacuates each PSUM bank to SBUF as soon as its two slots finish;
    the bank's output columns are then DMA'd to DRAM immediately
  - every instruction is emitted into one globally ordered stream
    (estimated-start-time heap gated by dependencies), because the Tile
    framework derives dependencies from program order

Cost-model notes that shaped this (from bass_rust instruction_cost_v2):
fp32 matmul = 4 PE cycles/row, float32r >= 256 rows = 1, bf16 = 1;
DVE 2x/4x perf modes apply only to the DVE engine, never Pool; LdWeights
is free; DMA is one exclusive device at ~360 GB/s aggregate, so per-core
HBM traffic (8.4 MB with bf16 inputs) floors DMA busy at ~23 us.
"""
import sys

for _p in ("/opt/trn_rl_repo",):
    if _p not in sys.path:
        sys.path.insert(0, _p)

from contextlib import ExitStack

import numpy as np

import concourse.bass as bass
import concourse.mybir as mybir
import concourse.tile as tile
from concourse import bacc
from concourse.bass_utils import run_bass_kernel_spmd

N_CORES = 8
P = 128
F32 = mybir.dt.float32
F32R = mybir.dt.float32r
BF16 = mybir.dt.bfloat16

_CACHE: dict = {}


def _detect_plan(r1, r2, ro, cg, in_dim, out_dim):
    """Return list of (s1, s2, so, c) slot terms, or None if the index tables
    don't have the aligned 128-run structure."""
    T = len(cg)
    if T % P != 0 or len(r1) != T or len(r2) != T or len(ro) != T:
        return None
    d1 = np.diff(r1)
    d2 = np.diff(r2)
    do = np.diff(ro)
    brk = np.where(~((d1 == 1) & (d2 == 1) & (do == 1)))[0] + 1
    starts = np.concatenate([[0], brk])
    ends = np.concatenate([brk, [T]])
    if not np.all(ends - starts == P):
        return None
    a0, b0, o0 = r1[starts], r2[starts], ro[starts]
    if (a0 % P).any() or (b0 % P).any() or (o0 % P).any():
        return None
    if a0.max() + P > in_dim or b0.max() + P > in_dim or o0.max() + P > out_dim:
        return None
    cg2 = np.asarray(cg).reshape(-1, P)
    if not np.all(cg2 == cg2[:, :1]):
        return None
    return list(
        zip(
            (a0 // P).tolist(),
            (b0 // P).tolist(),
            (o0 // P).tolist(),
            cg2[:, 0].astype(np.float64).tolist(),
        )
    )


def _numpy_fallback(x1, x2, cg, r1, r2, ro, out_dim):
    out = np.zeros((x1.shape[0], out_dim), dtype=x1.dtype)
    prod = x1[:, r1] * x2[:, r2] * cg[None, :].astype(x1.dtype)
    np.add.at(out, (slice(None), ro), prod)
    return out


# cost-model engine-busy estimates (ns) for [128, N]-free elementwise ops
def _dve_tt(free):  # bf16 tensor_tensor, 2x_1p (+ measured per-op overhead)
    return free * 1.0417 * 0.5 + 80.0


def _pool_tt(free):  # tensor_tensor; Pool gets no DVE 2x modes, 0.42 sw eff
    return free * 0.8333 / 0.42 + 30.0


def _dve_conv(free):  # fp32->bf16 tensor_copy, 2x_2p
    return free * 1.0417 * 0.5 + 60.0


def _act_conv(free):  # fp32->bf16 activation copy
    return free * 0.8333 + 185.0


def _pool_conv(free):  # fp32->bf16 copy on gpsimd (0.6 default sw efficiency)
    return free * 0.8333 / 0.6 + 30.0


_PLAN_CFG = {
    "act_vt0": 2200.0,
    "dve_conv_shadow": 1.0,
    "pool_conv": True,
    "work_conserve": False,  # producers: prefer the idle engine
    "act_conv_ps1": False,  # force pass>=1 conversions onto Act
    "chunks0": [2, 2, 2, 4, 4, 2],  # load chunk sizes (slots)
    "chunks1": [4, 4, 2, 2, 2, 2],  # later passes
    "act_conv_ps0_from": 3,  # pass-0 chunks >= this index convert on Act
    "n_combine": 10,  # mirrored pairs folded into S+- combines
    "act_conv_ps1_from": 99,  # pass>=1 chunks >= this index convert on Act
    "comb_offset": 2,  # skip the first N mirrored pairs when choosing combines
    "sid_spread": 40.0,  # est spacing between scaled-identity builds on Act
    "pool_evacs": 0,  # how many of the latest-finishing bank evacs go to Pool
    "n_combine_late": 0,  # also fold the latest-arriving mirrored pairs
}


_ACT_SID_NS = 292.0
_ACT_EVAC_NS = 612.0
_MM_NS = 107.0  # bf16 matmul, 256 moving rows

SLOTS_PER_GROUP = 4  # column-group granularity for input DMA (512 cols)


def _mirror_plan(pairs):
    """Split terms into direct terms and mirror-combined terms.

    Returns (direct, combined, combines) where
      direct:   list of (pair, so, c)             -> rhs = product(pair)
      combined: list of (upair, sign, so, c)      -> rhs = S_sign(upair)
      combines: list of (upair, sign)             -> S_sign = pr_ab + sign*pr_ba
    """
    direct, combined, combines = [], [], set()
    done = set()
    for (a, b), tl in pairs.items():
        if (a, b) in done:
            continue
        if a == b or (b, a) not in pairs:
            done.add((a, b))
            for so, c in tl:
                direct.append(((a, b), so, c))
            continue
        d1 = dict(tl)
        d2 = dict(pairs[(b, a)])
        done.add((a, b))
        done.add((b, a))
        if set(d1) != set(d2):
            for so, c in d1.items():
                direct.append(((a, b), so, c))
            for so, c in d2.items():
                direct.append(((b, a), so, c))
            continue
        ok = all(abs(abs(d1[so]) - abs(d2[so])) <= 1e-5 * abs(d1[so]) for so in d1)
        if not ok:
            for so, c in d1.items():
                direct.append(((a, b), so, c))
            for so, c in d2.items():
                direct.append(((b, a), so, c))
            continue
        up = (a, b) if a < b else (b, a)
        da, db = (d1, d2) if a < b else (d2, d1)
        for so in da:
            sign = 1 if da[so] * db[so] > 0 else -1
            combined.append((up, sign, so, da[so]))
            combines.add((up, sign))
    return direct, combined, sorted(combines)


def _build_program(terms, b_shard, in_dim, out_dim):
    """Build the per-core Bass program. Every core runs the same program on
    its own batch shard (data-parallel, no collectives).

    Slot-pass structure: all 4 batch row-blocks are in flight at once, so a
    PSUM bank holds exactly one output slot (512-wide moving dim) and each
    pair product is computed ONCE ([128, 512] bf16 tensor_tensor). The 16
    output slots run as two wavefronts of 8 banks; the second wave only
    waits for first-wave evacuations, not for new products.
    """
    nblk = b_shard // P
    assert nblk % 4 == 0 and nblk == 4, "slot-pass layout assumes 4 row-blocks"
    n_so = out_dim // P
    n_s_in = in_dim // P

    def chunks():
        sizes = _PLAN_CFG["chunks0"]
        out, s = [], 0
        for sz in sizes:
            if s >= n_s_in:
                break
            e = min(s + sz, n_s_in)
            out.append(list(range(s, e)))
            s = e
        while s < n_s_in:
            e = min(s + SLOTS_PER_GROUP, n_s_in)
            out.append(list(range(s, e)))
            s = e
        return out

    chunk_list = chunks()
    chunk_idx = {}
    for ci, chunk in enumerate(chunk_list):
        for s in chunk:
            chunk_idx[s] = ci

    # serial-DMA load completion estimates (bf16, 2x latency below 512 B)
    load_done = {}
    t = 1400.0
    for ci, chunk in enumerate(chunk_list):
        elem = len(chunk) * P * 2
        dur = nblk * P * elem / 0.36 * (2.0 if elem < 512 else 1.0)
        t += dur
        load_done[(0, ci)] = t
        t += dur
        load_done[(1, ci)] = t

    pairs: dict = {}
    for s1, s2, so, c in terms:
        pairs.setdefault((s1, s2), []).append((so, c))

    def grp_w(p):
        return max(p[0], p[1])

    direct_all, combined_all, _ = _mirror_plan(pairs)
    n_comb = _PLAN_CFG["n_combine"]
    upairs = sorted({up for up, sign, so, c in combined_all},
                    key=lambda up: max(grp_w(up), grp_w((up[1], up[0]))))
    off = _PLAN_CFG["comb_offset"]
    chosen = set(upairs[off : off + n_comb])
    direct = list(direct_all)
    combined, combines = [], set()
    for up, sign, so, c in combined_all:
        if up in chosen:
            combined.append((up, sign, so, c))
            combines.add((up, sign))
        else:
            direct.append((up, so, c))
            direct.append(((up[1], up[0]), so, c if sign > 0 else -c))
    combines = sorted(combines)

    # list-schedule products and combines on DVE/Pool (512-wide ops)
    import heapq as _hq

    FREE = nblk * P  # 512
    vt = {"dve": 0.0, "pool": 300.0}
    done = {}
    assign = {}
    comb_of_prod = {}
    for up, sign in combines:
        for pp in (up, (up[1], up[0])):
            comb_of_prod.setdefault(("prod", pp), []).append(("comb", up, sign))
    comb_deps = {("comb", up, sign): 2 for up, sign in combines}
    comb_ready = {k: 0.0 for k in comb_deps}

    heap = []
    seq = 0
    for p in pairs:
        ready = max(load_done[(0, chunk_idx[p[0]])], load_done[(1, chunk_idx[p[1]])])
        seq += 1
        _hq.heappush(heap, (ready, seq, ("prod", p)))
    while heap:
        ready, _, key = _hq.heappop(heap)
        cand = [
            ("dve", max(ready, vt["dve"]) + _dve_tt(FREE), _dve_tt(FREE)),
            ("pool", max(ready, vt["pool"]) + _pool_tt(FREE), _pool_tt(FREE)),
        ]
        eng, fin, cost = min(cand, key=lambda c: c[1])
        fin = max(ready, vt[eng]) + cost
        vt[eng] = fin
        assign[key] = eng
        done[key] = fin
        if key[0] == "prod":
            for w in comb_of_prod.get(key, []):
                comb_ready[w] = max(comb_ready[w], fin)
                comb_deps[w] -= 1
                if comb_deps[w] == 0:
                    seq += 1
                    _hq.heappush(heap, (comb_ready[w], seq, w))

    # slot plan: one wavefront ordering by last-rhs completion; first 8 slots
    # get fresh banks, the rest alias the earliest-evacuating banks
    rhs_of = {}
    for p, so, c in direct:
        rhs_of.setdefault(so, []).append((("prod", p), c))
    for up, sign, so, c in combined:
        rhs_of.setdefault(so, []).append((("comb", up, sign), c))
    key_of = {so: max(done[rk] for rk, _ in tl) for so, tl in rhs_of.items()}
    slot_order = sorted(rhs_of, key=lambda so: (key_of[so], so))
    n_banks = 8
    bank_of = {}
    evac_est = {}
    cvals_first_use = {}
    pe_vt = 0.0
    _MMW = FREE * 0.4167  # bf16 matmul ns at 512 moving rows
    for i, so in enumerate(slot_order):
        tl = sorted(rhs_of[so], key=lambda rc: done[rc[0]])
        rhs_of[so] = tl
        for rk, c in tl:
            pe_vt = max(pe_vt, done[rk]) + _MMW
            cvals_first_use.setdefault(c, len(cvals_first_use))
        bank_of[so] = (
            i if i < n_banks else bank_of[slot_order[i - n_banks]]
        )
        # monotone in slot_order: PE retires groups in exactly this order,
        # so this emission order never head-of-line-blocks the Act queue
        evac_est[so] = max(pe_vt + 100.0,
                           (evac_est[slot_order[i - 1]] + 1.0) if i else 0.0)

    # --- emit -------------------------------------------------------------
    nc = bacc.Bacc("TRN2", target_bir_lowering=False, debug=False)
    x1d = nc.dram_tensor("x1", [b_shard, in_dim], BF16, kind="ExternalInput").ap()
    x2d = nc.dram_tensor("x2", [b_shard, in_dim], BF16, kind="ExternalInput").ap()
    outd = nc.dram_tensor("out", [b_shard, out_dim], F32, kind="ExternalOutput").ap()

    with tile.TileContext(nc) as tc, ExitStack() as ctx:
        const_p = ctx.enter_context(tc.tile_pool(name="const", bufs=1))
        big_p = ctx.enter_context(tc.tile_pool(name="big", bufs=1))
        prod_p = ctx.enter_context(tc.tile_pool(name="prod", bufs=1))
        psum_p = ctx.enter_context(tc.tile_pool(name="psum", bufs=8, space="PSUM"))

        ident = const_p.tile([P, P], F32, tag="ident")
        nc.gpsimd.memset(ident[:], 0.0)
        nc.gpsimd.affine_select(
            out=ident[:],
            in_=ident[:],
            compare_op=mybir.AluOpType.not_equal,
            fill=1.0,
            base=0,
            pattern=[[-1, P]],
            channel_multiplier=1,
        )

        X1B = big_p.tile([P, nblk * in_dim], BF16, tag="X1B")
        X2B = big_p.tile([P, nblk * in_dim], BF16, tag="X2B")
        OUT = big_p.tile([P, nblk * out_dim], F32, tag="OUT")
        XBr = [
            X1B[:].rearrange("p (blk f) -> p blk f", blk=nblk),
            X2B[:].rearrange("p (blk f) -> p blk f", blk=nblk),
        ]
        OUTr = OUT[:].rearrange("p (blk f) -> p blk f", blk=nblk)

        banks = []
        for k in range(2 * n_banks):
            bk = psum_p.tile([P, FREE], F32, tag="bank")
            banks.append(bk)
        bank_tile = {}
        fresh = 0
        for i, so in enumerate(slot_order):
            bank_tile[so] = banks[i]  # pool rotation: i>=8 aliases i-8

        sids = {}
        for c, i in sorted(cvals_first_use.items(), key=lambda kv: kv[1]):
            t_ = const_p.tile([P, P], BF16, tag=f"sid{i}")
            sids[c] = t_

        raw_events = []

        def add(eid, est, deps, emit):
            raw_events.append((eid, est, deps, emit))

        for c, i in sorted(cvals_first_use.items(), key=lambda kv: kv[1]):
            def em_sid(c=c):
                nc.scalar.activation(
                    out=sids[c][:],
                    in_=ident[:],
                    func=mybir.ActivationFunctionType.Copy,
                    scale=float(c),
                )
            add(("sid", c), 500.0 + _PLAN_CFG["sid_spread"] * i, [], em_sid)

        for ci, chunk in enumerate(chunk_list):
            cols = slice(chunk[0] * P, (chunk[-1] + 1) * P)
            for tn, xd in ((0, x1d), (1, x2d)):
                elem = len(chunk) * P * 2
                dur = nblk * P * elem / 0.36 * (2.0 if elem < 512 else 1.0)
                def em_load(tn=tn, cols=cols, xd=xd):
                    nc.sync.dma_start(
                        out=XBr[tn][:, :, cols],
                        in_=xd[:, cols].rearrange("(blk p) f -> p blk f", p=P),
                    )
                add(("load", ci, tn), load_done[(tn, ci)] - dur, [], em_load)

        tiles = {}
        for p in pairs:
            key = ("prod", p)
            deps = [("load", chunk_idx[p[0]], 0), ("load", chunk_idx[p[1]], 1)]
            eng_name = assign[key]
            def em_prod(p=p, eng_name=eng_name, key=key):
                pr = prod_p.tile([P, FREE], BF16, tag="prod", bufs=96)
                eng = nc.vector if eng_name == "dve" else nc.gpsimd
                eng.tensor_tensor(
                    out=pr[:].rearrange("p (b f) -> p b f", b=nblk),
                    in0=XBr[0][:, :, p[0] * P : (p[0] + 1) * P],
                    in1=XBr[1][:, :, p[1] * P : (p[1] + 1) * P],
                    op=mybir.AluOpType.mult,
                )
                tiles[key] = pr
            add(key, done[key] - _dve_tt(FREE), deps, em_prod)

        for up, sign in combines:
            key = ("comb", up, sign)
            eng_name = assign[key]
            def em_comb(up=up, sign=sign, eng_name=eng_name, key=key):
                pr = prod_p.tile([P, FREE], BF16, tag="prod", bufs=96)
                eng = nc.vector if eng_name == "dve" else nc.gpsimd
                eng.tensor_tensor(
                    out=pr[:].rearrange("p (b f) -> p b f", b=nblk),
                    in0=tiles[("prod", up)][:].rearrange("p (b f) -> p b f", b=nblk),
                    in1=tiles[("prod", (up[1], up[0]))][:].rearrange(
                        "p (b f) -> p b f", b=nblk
                    ),
                    op=mybir.AluOpType.add if sign > 0 else mybir.AluOpType.subtract,
                )
                tiles[key] = pr
            add(key, done[key] - _dve_tt(FREE),
                [("prod", up), ("prod", (up[1], up[0]))], em_comb)

        for i, so in enumerate(slot_order):
            tl = rhs_of[so]
            deps = [rk for rk, _ in tl] + [("sid", c) for _, c in tl]
            if i >= n_banks:
                deps.append(("evac", slot_order[i - n_banks]))
            deps = sorted(set(deps))
            def em_slot(so=so, tl=tl):
                for j, (rk, c) in enumerate(tl):
                    nc.tensor.matmul(
                        out=bank_tile[so][:],
                        lhsT=sids[c][:],
                        rhs=tiles[rk][:],
                        start=(j == 0),
                        stop=(j == len(tl) - 1),
                    )
            add(("slot", so), key_of[so], deps, em_slot)

            def em_evac(so=so):
                nc.scalar.copy(
                    out=OUTr[:, :, so * P : (so + 1) * P],
                    in_=bank_tile[so][:].rearrange("p (b f) -> p b f", b=nblk),
                )
            add(("evac", so), evac_est[so], [("slot", so)], em_evac)

            store_est = max(evac_est[so] + 650.0, max(load_done.values()) + 1.0)
            def em_store(so=so):
                nc.sync.dma_start(
                    out=outd[:, so * P : (so + 1) * P].rearrange(
                        "(blk p) f -> p blk f", p=P
                    ),
                    in_=OUTr[:, :, so * P : (so + 1) * P],
                )
            add(("store", so), store_est, [("evac", so)], em_store)

        # topological emission in estimated-start order
        import heapq
        events = {}
        dependents = {}
        for eid, est, deps, emit in raw_events:
            events[eid] = {"est": est, "deps": [], "emit": emit}
        for eid, est, deps, emit in raw_events:
            for d in deps:
                assert d in events, (eid, d)
                events[eid]["deps"].append(d)
                dependents.setdefault(d, []).append(eid)
        ndeps = {eid: len(ev["deps"]) for eid, ev in events.items()}
        heap2 = []
        ctr = 0
        for eid, ev in events.items():
            if ndeps[eid] == 0:
                heapq.heappush(heap2, (ev["est"], ctr, eid))
                ctr += 1
        emitted = 0
        while heap2:
            _, _, eid = heapq.heappop(heap2)
            events[eid]["emit"]()
            emitted += 1
            for dep in dependents.get(eid, []):
                ndeps[dep] -= 1
                if ndeps[dep] == 0:
                    heapq.heappush(heap2, (events[dep]["est"], ctr, dep))
                    ctr += 1
        assert emitted == len(events), (emitted, len(events))

    nc.finalize()  # run the bacc pass pipeline (wait splitting, regalloc, ...)
    return nc


def kernel(x1, x2, cg_tilde, repids_in1, repids_in2, repids_out, out_dim):
    x1 = np.ascontiguousarray(np.asarray(x1, dtype=np.float32))
    x2 = np.ascontiguousarray(np.asarray(x2, dtype=np.float32))
    cg = np.asarray(cg_tilde, dtype=np.float32)
    r1 = np.asarray(repids_in1).astype(np.int64)
    r2 = np.asarray(repids_in2).astype(np.int64)
    ro = np.asarray(repids_out).astype(np.int64)
    out_dim = int(np.asarray(out_dim))

    B, in_dim = x1.shape
    terms = None
    if (
        B % (N_CORES * 2 * P) == 0
        and in_dim % P == 0
        and out_dim % P == 0
        and x2.shape == x1.shape
    ):
        terms = _detect_plan(r1, r2, ro, cg, in_dim, out_dim)
    if terms is None:
        return _numpy_fallback(x1, x2, cg, r1, r2, ro, out_dim)

    b_shard = B // N_CORES
    key = (B, in_dim, out_dim, np.asarray(terms, dtype=np.float64).tobytes())
    nc = _CACHE.get(key)
    if nc is None:
        nc = _build_program(terms, b_shard, in_dim, out_dim)
        _CACHE[key] = nc

    import ml_dtypes

    x1b = x1.astype(ml_dtypes.bfloat16)
    x2b = x2.astype(ml_dtypes.bfloat16)
    in_maps = [
        {
            "x1": x1b[i * b_shard : (i + 1) * b_shard],
            "x2": x2b[i * b_shard : (i + 1) * b_shard],
        }
        for i in range(N_CORES)
    ]
    res = run_bass_kernel_spmd(nc, in_maps, core_ids=list(range(N_CORES)))
    return np.concatenate([res.results[i]["out"] for i in range(N_CORES)], axis=0)
